# revision 43
# baseline (speedup 1.0000x reference)
"""GCN/GAT 4-layer GNN on 8 Trainium2 NeuronCores.

Strategy (vertex-cut data parallelism), v3:
  - dst-nodes sharded 8 ways (6250/core); each core owns all edges into its
    shard (host graph partitioning, dst-sorted, grouped into 128-dst blocks,
    blocks paired into gather GROUPS of 2).
  - Dense per-node matmuls on the owner core; per-layer gather tables
    (node features + src-side attention logits) AllGathered in TWO halves
    (split at local row 3200) so the first half of the collective overlaps
    the previous propagation tail; half-local row ids keep gather indices
    within int16.
  - Per-edge message passing: dma_gather fetches chunks of src rows into
    SBUF per 2-block group; each source half is split into two queue-
    parallel calls (4 SWDGE queues busy per group). The edge->dst_local
    one-hot AND its transpose are STATIC (graph-only), precomputed on the
    host in fp8 and streamed from DRAM (PE matmul takes fp8 stationary x
    bf16 moving); the one-hot turns segmented sums into PSUM-accumulated
    matmuls, the transposed one-hot fetches resident dst-side GAT logits
    (ed) via a tiny PSUM matmul.
  - GAT h columns are head-interleaved (j major, h minor) via host-side
    weight permutation so the per-edge softmax-weight multiply and the
    per-head normalization run in DVE 2x mode (packed bf16 last dim).
  - GAT softmax: exp without max-subtraction (logits are O(1)); exp
    weights (max(e^z, e^0.2z) = e^lrelu(z)) are written into the gathered
    rows' es slot so one fused matmul per chunk produces the aggregate and
    the softmax denominator. GCN norm: dis[src] folded into table rows,
    dis[dst] as postscale (scalar engine).
  - LayerNorm mostly on the scalar engine (per-partition bias/scale APs).
"""

import sys, os
for _p in ("/opt/trn_rl_repo", "/root/.axon_site/_ro/trn_rl_repo"):
    if os.path.isdir(_p) and _p not in sys.path:
        sys.path.insert(0, _p)

import numpy as np
import ml_dtypes

import concourse.bass as bass
import concourse.bacc as bacc
import concourse.mybir as mybir
import concourse.tile as tile
from concourse.bass_utils import run_bass_kernel_spmd

F32 = mybir.dt.float32
BF16 = mybir.dt.bfloat16
FP8 = mybir.dt.float8e4
I16 = mybir.dt.int16
BF = ml_dtypes.bfloat16
F8 = ml_dtypes.float8_e4m3fn
AF = mybir.ActivationFunctionType
OP = mybir.AluOpType

P = 8          # cores
HEADS = 8
NEG_SLOPE = 0.2
NUM_Q = 4
GS = 2         # blocks per gather group

# per-layer: row elems (gather granule, 256B-multiple), C = value cols,
# es at cols C..C+8 for GAT layers.
# L3 gathers fp8 rows: [h3 fp8 (512), es bf16-as-2xfp8 (16), pad]; other
# layers gather bf16 rows. `row` is in gather-dtype elements.
LAYERS = [
    dict(row=512, C=256, ch=32, gat=True, fp8=True),
    dict(row=128, C=128, ch=None, gat=False, fp8=False),
    dict(row=768, C=512, ch=64, gat=True, fp8=True),
    dict(row=128, C=64, ch=None, gat=False, fp8=False),
]


def _wrap_chunk(idx128):
    """Wrap one 128-idx chunk into dma_gather layout [128, 8] i16.
    Positions beyond len(idx128) gather row 0."""
    arr = np.zeros((16, 8), np.int16)
    n = len(idx128)
    for off in range(16):
        sub = idx128[off::16]
        arr[off, : len(sub)] = sub
    return np.tile(arr, (8, 1))


def preprocess(edge_index, N):
    """Graph partitioning + per-core index metadata with half-local row
    tables, 2-block gather groups, and static one-hot matrices."""
    NSH = N // P
    NB = (NSH + 127) // 128
    NBA = min((NB + 1) // 2, NB)
    HA = min(NSH, NBA * 128)          # rows in half A (blocks 0..NBA-1)
    HB = NSH - HA

    src = np.concatenate([np.asarray(edge_index[0]), np.arange(N)]).astype(np.int64)
    dst = np.concatenate([np.asarray(edge_index[1]), np.arange(N)]).astype(np.int64)
    deg = np.bincount(dst, minlength=N).astype(np.float64)
    dis = (1.0 / np.sqrt(np.maximum(deg, 1e-12))).astype(np.float32)

    core = dst // NSH
    blk = (dst % NSH) // 128
    rsrc = src % NSH
    csrc = src // NSH
    hi = (rsrc >= HA).astype(np.int64)
    gidx = np.where(hi == 0, csrc * HA + rsrc, csrc * HB + (rsrc - HA))

    counts = np.zeros((P, NB, 2), np.int64)
    np.add.at(counts, (core, blk, hi), 1)
    K = np.maximum(1, -(-counts // 128)).max(axis=0)  # [NB, 2] chunks per half

    # groups: B-section blocks (NBA..NB-1) processed first, then A-section
    # (0..NBA-1); each section paired into groups of GS blocks.
    secB = list(range(NBA, NB))
    secA = list(range(NBA))
    groups = [secB[i:i + GS] for i in range(0, len(secB), GS)] + \
             [secA[i:i + GS] for i in range(0, len(secA), GS)]

    # chunk layout per group: [Bsrc(b1) Bsrc(b2) | Asrc(b1) Asrc(b2)]
    # (source-half major so each half is one contiguous gather range).
    chunk_order = []   # list of (block, half, chunk_within)
    gmeta = []
    for g in groups:
        c0 = len(chunk_order)
        for h in (1, 0):  # B-source chunks first
            for b in g:
                for c in range(int(K[b, h])):
                    chunk_order.append((b, h, c))
        kb = sum(int(K[b, 1]) for b in g)
        ka = sum(int(K[b, 0]) for b in g)
        gmeta.append(dict(blocks=g, c0=c0, kb=kb, ka=ka))
    tot_ch = len(chunk_order)

    # per-block chunk ids (group order: B chunks then A chunks)
    blk_chunks = {b: [] for b in range(NB)}
    for ci, (b, h, c) in enumerate(chunk_order):
        blk_chunks[b].append(ci)

    order = np.lexsort((hi, blk, core))
    so_g, so_dst, so_core, so_blk, so_hi = (a[order] for a in (gidx, dst, core, blk, hi))

    percore = []
    for cc in range(P):
        m = so_core == cc
        cg, cdst = so_g[m], so_dst[m]
        cblk, chi = so_blk[m], so_hi[m]
        # per (block, half): index array + local-dst array
        per_bh = {}
        ptr = 0
        for b in range(NB):
            for h in range(2):
                n_e = int(counts[cc, b, h])
                per_bh[(b, h)] = (cg[ptr: ptr + n_e],
                                  (cdst[ptr: ptr + n_e] - cc * NSH - b * 128))
                ptr += n_e
        assert ptr == m.sum()
        # walk chunk_order: build sidx columns + dloc per chunk
        sw = []
        dloc = np.full((tot_ch, 128), 200, np.int64)
        for ci, (b, h, c) in enumerate(chunk_order):
            gi, dl = per_bh[(b, h)]
            a0, a1 = c * 128, min((c + 1) * 128, len(gi))
            if a1 > a0:
                sw.append(_wrap_chunk(gi[a0:a1]))
                dloc[ci, : a1 - a0] = dl[a0:a1]
            else:
                sw.append(_wrap_chunk(gi[0:0]))
        sidx = np.concatenate(sw, axis=1).astype(np.int16)
        dvals = np.arange(128, dtype=np.int64)
        # one-hot [e, (chunk, d)] and transposed [d, (chunk, e)] (fp8 exact)
        oh = (dloc[:, :, None] == dvals[None, None, :])          # [ch, e, d]
        oh_t = oh.transpose(1, 0, 2).reshape(128, tot_ch * 128)  # [e, (ch, d)]
        ohT_t = oh.transpose(2, 0, 1).reshape(128, tot_ch * 128)  # [d, (ch, e)]
        percore.append(dict(
            sidx=sidx,
            oh=oh_t.astype(F8),
            ohT=ohT_t.astype(F8),
            dis=dis[cc * NSH: (cc + 1) * NSH],
        ))
    sched = dict(N=N, NSH=NSH, NB=NB, NBA=NBA, HA=HA, HB=HB, K=K,
                 tot_ch=tot_ch, gmeta=gmeta, blk_chunks=blk_chunks)
    return sched, percore


def _perm(ch):
    """column permutation: new (j,h)-interleaved position j*8+h <- old h*ch+j"""
    p = np.empty(ch * HEADS, np.int64)
    for j in range(ch):
        for h in range(HEADS):
            p[j * HEADS + h] = h * ch + j
    return p


def prep_weights(w, N):
    """Host-side weight packing (bf16) shared by all cores."""
    def b16(a):
        return np.asarray(a, np.float32).astype(BF)

    p1 = _perm(32)
    p3 = _perm(64)

    W1 = np.asarray(w["g1_W"], np.float32)
    ws1s = np.einsum("khj,hj->kh", W1.reshape(1024, 8, 32), np.asarray(w["g1_as"], np.float32))
    ws1d = np.einsum("khj,hj->kh", W1.reshape(1024, 8, 32), np.asarray(w["g1_ad"], np.float32))
    m1W = np.asarray(w["m1_W"], np.float32)
    w1cat = np.concatenate([W1[:, p1], m1W[:, p1], ws1s, ws1d], axis=1)  # [1024, 528]
    w1t = b16(w1cat).reshape(8, 128, 528).transpose(1, 0, 2).copy()  # [128, 8, 528]

    # x1 columns are (j,h)-permuted -> permute W2/m2 rows to match
    w2cat = np.concatenate([np.asarray(w["g2_W"], np.float32),
                            np.asarray(w["m2_W"], np.float32)], axis=1)[p1]
    w2t = b16(w2cat).reshape(2, 128, 256).transpose(1, 0, 2).copy()  # [128, 2, 256]

    W3 = np.asarray(w["g3_W"], np.float32)
    ws3s = np.einsum("khj,hj->kh", W3.reshape(128, 8, 64), np.asarray(w["g3_as"], np.float32))
    ws3d = np.einsum("khj,hj->kh", W3.reshape(128, 8, 64), np.asarray(w["g3_ad"], np.float32))
    w3t = b16(np.concatenate([W3[:, p3], np.asarray(w["m3_W"], np.float32),
                              ws3s, ws3d], axis=1))  # [128, 592]

    w4t = b16(np.concatenate([np.asarray(w["g4_W"], np.float32),
                              np.asarray(w["m4_W"], np.float32)], axis=1))  # [64, 4]

    def rep(v):
        return np.tile(np.asarray(v, np.float32)[None, :], (128, 1)).copy()

    b1 = (np.asarray(w["g1_b"]) + np.asarray(w["m1_b"]))[p1]
    return dict(
        w1=w1t, w2=w2t, w3=w3t, w4=w4t,
        bias4=rep(np.asarray(w["g4_b"]) + np.asarray(w["m4_b"])),
        ident=np.eye(128, dtype=np.float32).astype(BF),
        onesr=np.ones((1, 128), dtype=np.float32).astype(BF),
        b1r=np.asarray(b1, np.float32)[None, :].astype(BF),
        b2r=np.asarray(w["g2_b"] + w["m2_b"], np.float32)[None, :].astype(BF),
        b3r=np.asarray(w["g3_b"] + w["m3_b"], np.float32)[None, :].astype(BF),
    )


def bcast(ap, count):
    """Append a 0-stride broadcast dim of `count` to an AP."""
    return bass.AP(ap.tensor, ap.offset, list(ap.ap) + [[0, count]])


def flat_view(t_ap, c0, rowstride, a, b, nch=None):
    """View chunk-major flat tile [128, W] as rows: [128, nch, b-a] starting
    at chunk c0, row stride `rowstride`, cols [a, b)."""
    p0 = t_ap.ap[0]
    if nch is None:
        return bass.AP(t_ap.tensor, t_ap.offset + c0 * rowstride + a,
                       [p0, [1, b - a]])
    return bass.AP(t_ap.tensor, t_ap.offset + c0 * rowstride + a,
                   [p0, [rowstride, nch], [1, b - a]])


def build_nc(sched):
    N, NSH, NB, NBA = sched["N"], sched["NSH"], sched["NB"], sched["NBA"]
    HA, HB, K, tot_ch = sched["HA"], sched["HB"], sched["K"], sched["tot_ch"]
    gmeta, blk_chunks = sched["gmeta"], sched["blk_chunks"]
    ROWMAX = max(c["row"] for c in LAYERS)
    KG = max(g["kb"] + g["ka"] for g in gmeta)       # max chunks per group
    KS = int(np.asarray(K).max()) + 1                # max chunks per block-half
    KB_MAX = max(len(v) for v in blk_chunks.values())  # max chunks per block

    nc = bacc.Bacc("TRN2", target_bir_lowering=False, debug=False, num_devices=P,
                   num_swdge_queues=NUM_Q)

    # ---- I/O ----
    xT_in = nc.dram_tensor("xT", [NB, 128, 1024], BF16, kind="ExternalInput")
    w1_in = nc.dram_tensor("w1", [128, 8, 528], BF16, kind="ExternalInput")
    w2_in = nc.dram_tensor("w2", [128, 2, 256], BF16, kind="ExternalInput")
    w3_in = nc.dram_tensor("w3", [128, 592], BF16, kind="ExternalInput")
    w4_in = nc.dram_tensor("w4", [64, 4], BF16, kind="ExternalInput")
    b4_in = nc.dram_tensor("bias4", [128, 2], F32, kind="ExternalInput")
    id_in = nc.dram_tensor("ident", [128, 128], BF16, kind="ExternalInput")
    onesr_in = nc.dram_tensor("onesr", [1, 128], BF16, kind="ExternalInput")
    b1r_in = nc.dram_tensor("b1r", [1, 256], BF16, kind="ExternalInput")
    b2r_in = nc.dram_tensor("b2r", [1, 128], BF16, kind="ExternalInput")
    b3r_in = nc.dram_tensor("b3r", [1, 64], BF16, kind="ExternalInput")
    dis_in = nc.dram_tensor("dis", [128, NB], F32, kind="ExternalInput")
    sidx_in = nc.dram_tensor("sidx", [128, tot_ch * 8], I16, kind="ExternalInput")
    oh_in = nc.dram_tensor("oh", [128, tot_ch * 128], FP8, kind="ExternalInput")
    ohT_in = nc.dram_tensor("ohT", [128, tot_ch * 128], FP8, kind="ExternalInput")
    out_dram = nc.dram_tensor("out", [NSH, 2], F32, kind="ExternalOutput")

    # ---- internal DRAM: bounce shards (two halves) + allgathered tables ----
    mlp0_dram = nc.dram_tensor("mlp0d", [NB, 128, 256], BF16)
    mlp1_dram = nc.dram_tensor("mlp1d", [NB, 128, 128], BF16)
    rows = [c["row"] for c in LAYERS]
    tdt = [FP8 if c["fp8"] else BF16 for c in LAYERS]
    TbA = [nc.dram_tensor(f"T{l+1}bA", [HA, rows[l]], tdt[l]) for l in range(4)]
    TbB = [nc.dram_tensor(f"T{l+1}bB", [HB, rows[l]], tdt[l]) for l in range(4)]
    TgA = [nc.dram_tensor(f"T{l+1}A", [P * HA, rows[l]], tdt[l], addr_space="Shared")
           for l in range(4)]
    TgB = [nc.dram_tensor(f"T{l+1}B", [P * HB, rows[l]], tdt[l], addr_space="Shared")
           for l in range(4)]

    blk_rows = [min(128, NSH - b * 128) for b in range(NB)]

    gq = [0]  # gather queue round-robin
    pref = {}

    def next_q():
        q = gq[0]
        gq[0] = (q + 1) % NUM_Q
        return q

    with tile.TileContext(nc) as tc:
        with (
            tc.tile_pool(name="consts", bufs=1) as cpool,
            tc.tile_pool(name="resident", bufs=1) as rpool,
            tc.tile_pool(name="meta", bufs=1) as mpool,
            tc.tile_pool(name="xload", bufs=2) as xpool,
            tc.tile_pool(name="gbig", bufs=2) as gpool,
            tc.tile_pool(name="gsmall", bufs=3) as spool,
            tc.tile_pool(name="prod", bufs=2) as ppool,
            tc.tile_pool(name="ohp", bufs=3) as opool,
            tc.tile_pool(name="ohtp", bufs=2) as tpool,
            tc.tile_pool(name="work", bufs=2) as wpool,
            tc.tile_pool(name="asm", bufs=2) as apool,
            tc.tile_pool(name="psA", bufs=2, space="PSUM") as psA,
            tc.tile_pool(name="psS", bufs=2, space="PSUM") as psS,
            tc.tile_pool(name="psT", bufs=2, space="PSUM") as psT,
            tc.tile_pool(name="psE", bufs=2, space="PSUM") as psE,
        ):
            # ---------- constants ----------
            w1_t = cpool.tile([128, 8, 528], BF16); nc.sync.dma_start(w1_t[:], w1_in[:])
            w2_t = cpool.tile([128, 2, 256], BF16); nc.sync.dma_start(w2_t[:], w2_in[:])
            w3_t = cpool.tile([128, 592], BF16); nc.sync.dma_start(w3_t[:], w3_in[:])
            w4_t = cpool.tile([64, 4], BF16); nc.sync.dma_start(w4_t[:], w4_in[:])
            b4_t = cpool.tile([128, 2], F32); nc.sync.dma_start(b4_t[:], b4_in[:])
            id_t = cpool.tile([128, 128], BF16); nc.sync.dma_start(id_t[:], id_in[:])
            ones_t = cpool.tile([1, 128], BF16); nc.sync.dma_start(ones_t[:], onesr_in[:])
            b1r_t = cpool.tile([1, 256], BF16); nc.sync.dma_start(b1r_t[:], b1r_in[:])
            b2r_t = cpool.tile([1, 128], BF16); nc.sync.dma_start(b2r_t[:], b2r_in[:])
            b3r_t = cpool.tile([1, 64], BF16); nc.sync.dma_start(b3r_t[:], b3r_in[:])
            dis_t = cpool.tile([128, NB], F32); nc.sync.dma_start(dis_t[:], dis_in[:])
            zcol = cpool.tile([128, 1], F32); nc.vector.memset(zcol[:], 0.0)
            epscol = cpool.tile([128, 1], F32); nc.vector.memset(epscol[:], 1e-5)
            sidx_t = mpool.tile([128, tot_ch * 8], I16); nc.sync.dma_start(sidx_t[:], sidx_in[:])

            mlp2_sb = rpool.tile([128, NB, 64], BF16, name="mlp2_sb", tag="mlp2_sb")
            x3_sb = rpool.tile([128, NB, 64], BF16)
            edl1 = rpool.tile([128, NB, 8], BF16)
            edl3 = rpool.tile([128, NB, 8], BF16)

            # one-time zero of gather buffers (stale tails of short groups are
            # never read, but first-use NaN garbage would trip sim checks)
            for _ in range(2):
                g = gpool.tile([128, KG * ROWMAX], FP8, tag="G")
                nc.vector.memset(g[:], 0.0)
            for _ in range(3):
                g = spool.tile([128, KG * 128], BF16, tag="Gs")
                nc.vector.memset(g[:], 0.0)

            def tb_write(l, b, src_ap):
                r = blk_rows[b]
                if b < NBA:
                    nc.sync.dma_start(TbA[l][b * 128: b * 128 + r, :], src_ap[0:r, :])
                else:
                    r0 = b * 128 - HA
                    nc.sync.dma_start(TbB[l][r0: r0 + r, :], src_ap[0:r, :])

            def ag(l, half):
                src, dst = (TbA[l], TgA[l]) if half == 0 else (TbB[l], TgB[l])
                nc.gpsimd.collective_compute(
                    "AllGather", OP.bypass, replica_groups=[list(range(P))],
                    ins=[src.ap().opt()], outs=[dst.ap().opt()])

            order_blocks = list(range(NBA, NB)) + list(range(NBA))

            # ---------- dense phase 1: h1|mlp1|es1|ed1 from x ----------
            for b in order_blocks:
                xt = xpool.tile([128, 1024], BF16)
                nc.sync.dma_start(xt[:], xT_in[b])
                pd = psA.tile([128, 512], F32, tag="psA")
                pe = psS.tile([128, 16], F32, tag="psS")
                for k in range(8):
                    nc.tensor.matmul(pd[:], xt[:, k * 128:(k + 1) * 128],
                                     w1_t[:, k, 0:512], start=(k == 0), stop=(k == 7))
                nc.tensor.matmul(pd[:, 256:512], ones_t[:], b1r_t[:],
                                 start=False, stop=True, skip_group_check=True)
                for k in range(8):
                    nc.tensor.matmul(pe[:], xt[:, k * 128:(k + 1) * 128],
                                     w1_t[:, k, 512:528], start=(k == 0), stop=(k == 7))
                as1 = apool.tile([128, 512], FP8, tag="as1")
                nc.scalar.copy(as1[:, 0:256], pd[:, 0:256])
                nc.scalar.copy(as1[:][:, 256:272].bitcast(BF16), pe[:, 0:8])
                tb_write(0, b, as1)
                nc.scalar.copy(edl1[:, b, :], pe[:, 8:16])
                m0 = apool.tile([128, 256], BF16, tag="m0")
                nc.scalar.copy(m0[:], pd[:, 256:512])
                nc.sync.dma_start(mlp0_dram[b], m0[:])
                if b == NB - 1:
                    ag(0, 1)
            ag(0, 0)

            # ---------- propagation ----------
            def prop(l, epilogue):
                cfg = LAYERS[l - 1]
                row, C, gat = cfg["row"], cfg["C"], cfg["gat"]
                fp8 = cfg["fp8"]
                edl = edl1 if l == 1 else (edl3 if l == 3 else None)
                pool = gpool if row > 128 else spool
                gtag = "G" if row > 128 else "Gs"
                gwidth = KG * row
                for gi, gm in enumerate(gmeta):
                    blocks, c0, kb, ka = gm["blocks"], gm["c0"], gm["kb"], gm["ka"]
                    kt = kb + ka
                    g = pool.tile([128, gwidth], FP8 if fp8 else BF16, tag=gtag)

                    def issue(tbl, ch0, ch1):
                        # gather chunks [ch0, ch1) of this group from table
                        nch = ch1 - ch0
                        if nch <= 0:
                            return
                        nc.gpsimd.dma_gather(
                            out_ap=flat_view(g[:], ch0, row, 0, row, nch),
                            in_ap=tbl[:],
                            idxs_ap=sidx_t[:, (c0 + ch0) * 8: (c0 + ch1) * 8],
                            num_idxs=nch * 128, num_idxs_reg=nch * 128,
                            elem_size=row, single_packet=False,
                            queue_num=(next_q() % 2) + (0 if row > 128 else 2))

                    issue(TgB[l - 1], 0, kb // 2)
                    issue(TgB[l - 1], kb // 2, kb)
                    issue(TgA[l - 1], kb, kb + (ka + 1) // 2)
                    issue(TgA[l - 1], kb + (ka + 1) // 2, kt)

                    # static one-hot + transposed one-hot for this group
                    oh_t = opool.tile([128, KG * 128], FP8, tag="oh")
                    nc.sync.dma_start(oh_t[:, 0: kt * 128],
                                      oh_in[:, (c0) * 128: (c0 + kt) * 128])
                    if gat:
                        ohT_t = tpool.tile([128, KG * 128], FP8, tag="ohT")
                        nc.sync.dma_start(ohT_t[:, 0: kt * 128],
                                          ohT_in[:, (c0) * 128: (c0 + kt) * 128])

                    if l == 1:
                        for b in blocks:
                            m0p = wpool.tile([128, 256], BF16, tag="m0l")
                            nc.sync.dma_start(m0p[:], mlp0_dram[b])
                            pref[b] = m0p
                    elif l == 2:
                        for b in blocks:
                            m1p = wpool.tile([128, 128], BF16, tag="m1l")
                            nc.sync.dma_start(m1p[:], mlp1_dram[b])
                            pref[b] = m1p

                    for b in blocks:
                        chunks = [ci - c0 for ci in blk_chunks[b]]
                        nchb = len(chunks)

                        def gview(c, a2, b2, n=None):
                            return flat_view(g[:], c, row, a2, b2, n)

                        def ohv(c):
                            return bass.AP(oh_t[:].tensor, oh_t[:].offset + c * 128,
                                           [oh_t[:].ap[0], [1, 128]])

                        pagg = psA.tile([128, 264 if (gat and C == 256) else C],
                                        F32, tag="psA")
                        if gat:
                            ch = cfg["ch"]
                            W = C + 8   # product row: [h*w (C), w (8)] bf16
                            # per-edge dst logits via transposed one-hot
                            pse = psE.tile([128, KG, 8], F32, tag="psE")
                            for c in chunks:
                                nc.tensor.matmul(
                                    pse[:, c, :],
                                    bass.AP(ohT_t[:].tensor, ohT_t[:].offset + c * 128,
                                            [ohT_t[:].ap[0], [1, 128]]),
                                    edl[:, b, :], start=True, stop=True,
                                    skip_group_check=True)
                            prod = ppool.tile([128, KB_MAX * W], BF16, tag="prod")
                            # chunks of this block form contiguous spans
                            # [B-part][A-part] (split at KS for tile sizing)
                            spans = []
                            s0 = chunks[0]
                            prev = s0
                            for c in chunks[1:]:
                                if c != prev + 1 or c - s0 >= KS:
                                    spans.append((s0, prev + 1)); s0 = c
                                prev = c
                            spans.append((s0, prev + 1))
                            pos0 = 0
                            for (sa, sb) in spans:
                                nsp = sb - sa
                                # es (bf16; stored as raw bytes in fp8 rows)
                                if fp8:
                                    esv = bass.AP(g[:].tensor, g[:].offset + sa * row + C,
                                                  [g[:].ap[0], [row, nsp], [1, 16]]
                                                  ).bitcast(BF16)
                                else:
                                    esv = gview(sa, C, C + 8, nsp)
                                tsum = wpool.tile([128, KS, 8], BF16, tag="tsum")
                                nc.vector.tensor_tensor(
                                    tsum[:, 0:nsp, :], esv,
                                    pse[:, sa:sb, :], OP.add)
                                e1 = wpool.tile([128, KS, 8], BF16, tag="e1")
                                nc.scalar.activation(e1[:, 0:nsp, :], tsum[:, 0:nsp, :],
                                                     AF.Exp, bias=zcol[:])
                                e5 = wpool.tile([128, KS, 8], BF16, tag="e5")
                                nc.scalar.activation(e5[:, 0:nsp, :], tsum[:, 0:nsp, :],
                                                     AF.Exp, bias=zcol[:], scale=NEG_SLOPE)
                                # exp(lrelu(z)) = max(e^z, e^.2z) -> prod w slot
                                wslot = bass.AP(prod[:].tensor,
                                                prod[:].offset + pos0 * W + C,
                                                [prod[:].ap[0], [W, nsp], [1, 8]])
                                nc.vector.tensor_tensor(wslot, e1[:, 0:nsp, :],
                                                        e5[:, 0:nsp, :], OP.max)
                                # h (fp8, head-interleaved) * w -> prod (bf16)
                                h4 = bass.AP(g[:].tensor, g[:].offset + sa * row,
                                             [g[:].ap[0], [row, nsp], [HEADS, ch], [1, HEADS]])
                                pv = bass.AP(prod[:].tensor,
                                             prod[:].offset + pos0 * W,
                                             [prod[:].ap[0], [W, nsp], [HEADS, ch], [1, HEADS]])
                                exb = bass.AP(prod[:].tensor,
                                              prod[:].offset + pos0 * W + C,
                                              [prod[:].ap[0], [W, nsp], [0, ch], [1, HEADS]])
                                nc.vector.tensor_tensor(pv, h4, exb, OP.mult)
                                pos0 += nsp
                            # fused aggregate + denominator matmuls over prod
                            def pview(pos, a2, b2):
                                return bass.AP(prod[:].tensor,
                                               prod[:].offset + pos * W + a2,
                                               [prod[:].ap[0], [1, b2 - a2]])
                            if C == 256:
                                for i, c in enumerate(chunks):
                                    nc.tensor.matmul(pagg[:], ohv(c), pview(i, 0, 264),
                                                     start=(i == 0), stop=(i == nchb - 1),
                                                     skip_group_check=True)
                                den = pagg[:, 256:264]
                                val = pagg[:, 0:256]
                            else:
                                pden = psE.tile([128, 8], F32, tag="psE")
                                for i, c in enumerate(chunks):
                                    nc.tensor.matmul(pagg[:], ohv(c), pview(i, 0, 512),
                                                     start=(i == 0), stop=(i == nchb - 1),
                                                     skip_group_check=True)
                                for i, c in enumerate(chunks):
                                    nc.tensor.matmul(pden[:], ohv(c), pview(i, 512, 520),
                                                     start=(i == 0), stop=(i == nchb - 1),
                                                     skip_group_check=True)
                                den = pden[:]
                                val = pagg[:, 0:512] if C == 512 else pagg[:, 0:256]
                            sden = wpool.tile([128, 8], F32, tag="sden")
                            nc.vector.tensor_scalar(sden[:], den, 1e-16, None, OP.add)
                            rs = wpool.tile([128, 8], F32, tag="rs")
                            nc.vector.reciprocal(rs[:], sden[:])
                            if l == 3:  # fold the 1/heads of the head-mean in
                                nc.vector.tensor_scalar(rs[:], rs[:], 1.0 / HEADS, None, OP.mult)
                            agf = wpool.tile([128, C], BF16, tag="agf")
                            nc.vector.tensor_tensor(
                                bass.AP(agf[:].tensor, agf[:].offset,
                                        [agf[:].ap[0], [HEADS, ch], [1, HEADS]]),
                                bass.AP(val.tensor, val.offset,
                                        [val.ap[0], [HEADS, ch], [1, HEADS]]),
                                bass.AP(rs[:].tensor, rs[:].offset,
                                        [rs[:].ap[0], [0, ch], [1, HEADS]]),
                                OP.mult)
                            epilogue(b, agf)
                        else:
                            for i, c in enumerate(chunks):
                                nc.tensor.matmul(pagg[:], ohv(c), gview(c, 0, C),
                                                 start=(i == 0), stop=(i == nchb - 1),
                                                 skip_group_check=True)
                            agf = wpool.tile([128, C], BF16, tag="agf")
                            nc.scalar.mul(agf[:], pagg[:], dis_t[:, b: b + 1])
                            epilogue(b, agf)
                        if b == NB - 1 and l < 4:
                            ag(l, 1)
                if l < 4:
                    ag(l, 0)

            def layer_norm(t, Cn):
                """LN over free dim (ln weight=1 bias=0); scalar-engine heavy.
                In-place: t is centered and scaled; returns bf16 tile."""
                mu = wpool.tile([128, 1], F32, tag="mu")
                nc.vector.tensor_reduce(mu[:], t[:], mybir.AxisListType.X, OP.add)
                mun = wpool.tile([128, 1], F32, tag="mun")
                nc.scalar.mul(mun[:], mu[:], -1.0 / Cn)
                nc.scalar.activation(t[:], t[:], AF.Identity, bias=mun[:])
                sq = wpool.tile([128, Cn], BF16, tag="sq")
                vs = wpool.tile([128, 1], F32, tag="vs")
                nc.scalar.activation(sq[:], t[:], AF.Square, bias=zcol[:], accum_out=vs[:])
                sd = wpool.tile([128, 1], F32, tag="sd")
                nc.scalar.activation(sd[:], vs[:], AF.Sqrt, bias=epscol[:], scale=1.0 / Cn)
                rstd = wpool.tile([128, 1], F32, tag="rstd")
                nc.vector.reciprocal(rstd[:], sd[:])
                xo = wpool.tile([128, Cn], BF16, tag="xo")
                nc.scalar.mul(xo[:], t[:], rstd[:])
                return xo

            def transpose_to(x_ap, cols):
                pt = psT.tile([128, 128], BF16, tag="psT")
                nc.tensor.transpose(pt[0:cols, :], x_ap, id_t[:])
                xt_ = wpool.tile([cols, 128], BF16, tag=f"tr{cols}")
                nc.scalar.copy(xt_[:], pt[0:cols, :])
                return xt_

            # ---- L1 epilogue: LN -> x1, dense-2 (h2'|mlp2), T2 assembly ----
            def epi1(b, agf):
                x1p = wpool.tile([128, 256], BF16, tag="x1p")
                nc.vector.tensor_tensor(x1p[:], agf[:], pref.pop(b)[:], OP.add)
                x1 = layer_norm(x1p, 256)
                xta = transpose_to(x1[:, 0:128], 128)
                xtb = transpose_to(x1[:, 128:256], 128)
                ps2 = psS.tile([128, 256], F32, tag="psS")
                nc.tensor.matmul(ps2[:], xta[:], w2_t[:, 0, :], start=True, stop=False)
                nc.tensor.matmul(ps2[:], xtb[:], w2_t[:, 1, :], start=False, stop=True)
                nc.tensor.matmul(ps2[:, 128:256], ones_t[:], b2r_t[:],
                                 start=False, stop=True, skip_group_check=True)
                as2 = apool.tile([128, 128], BF16, tag="as2")
                nc.scalar.mul(as2[:], ps2[:, 0:128], dis_t[:, b: b + 1])
                tb_write(1, b, as2)
                m1 = apool.tile([128, 128], BF16, tag="m1")
                nc.scalar.copy(m1[:], ps2[:, 128:256])
                nc.sync.dma_start(mlp1_dram[b], m1[:])

            # ---- L2 epilogue: LN -> x2, dense-3, T3 assembly ----
            def epi2(b, agf):
                x2p = wpool.tile([128, 128], BF16, tag="x2p")
                nc.vector.tensor_tensor(x2p[:], agf[:], pref.pop(b)[:], OP.add)
                x2 = layer_norm(x2p, 128)
                xt2 = transpose_to(x2[:], 128)
                ps3a = psA.tile([128, 512], F32, tag="psA")
                nc.tensor.matmul(ps3a[:], xt2[:], w3_t[:, 0:512], start=True, stop=True)
                ps3b = psS.tile([128, 80], F32, tag="psS")
                nc.tensor.matmul(ps3b[:], xt2[:], w3_t[:, 512:592], start=True, stop=False)
                nc.tensor.matmul(ps3b[:, 0:64], ones_t[:], b3r_t[:],
                                 start=False, stop=True, skip_group_check=True)
                as3 = apool.tile([128, 768], FP8, tag="as3")
                nc.scalar.copy(as3[:, 0:512], ps3a[:])
                nc.scalar.copy(as3[:][:, 512:528].bitcast(BF16), ps3b[:, 64:72])
                tb_write(2, b, as3)
                nc.scalar.copy(edl3[:, b, :], ps3b[:, 72:80])
                nc.scalar.copy(mlp2_sb[:, b, :], ps3b[:, 0:64])

            # ---- L3 epilogue: mean heads (1/8 folded into rs), LN -> x3 ----
            def epi3(b, agf):
                mf = wpool.tile([128, 64], BF16, tag="mf")
                a = agf[:]
                with nc.allow_low_precision(reason="mean of 8 bf16 head slices"):
                    nc.vector.tensor_reduce(
                        mf[:], bass.AP(a.tensor, a.offset, [a.ap[0], [HEADS, 64], [1, HEADS]]),
                        mybir.AxisListType.X, OP.add)
                t = wpool.tile([128, 64], BF16, tag="t3")
                nc.vector.tensor_tensor(t[:], mf[:], mlp2_sb[:, b, :], OP.add)
                x3 = layer_norm(t, 64)
                nc.scalar.copy(x3_sb[:, b, :], x3[:])
                as4 = apool.tile([128, 128], BF16, tag="as4")
                nc.scalar.mul(as4[:, 0:64], x3[:], dis_t[:, b: b + 1])
                tb_write(3, b, as4)

            # ---- L4 epilogue: (agg @ W4) + (x3 @ m4_W) + bias ----
            def epi4(b, agf):
                a4T = transpose_to(agf[:], 64)
                x3T = transpose_to(x3_sb[:, b, :], 64)
                ps4 = psS.tile([128, 2], F32, tag="psS")
                nc.tensor.matmul(ps4[:], a4T[:], w4_t[:, 0:2], start=True, stop=False)
                nc.tensor.matmul(ps4[:], x3T[:], w4_t[:, 2:4], start=False, stop=True)
                ot = wpool.tile([128, 2], F32, tag="ot")
                nc.vector.tensor_tensor(ot[:], ps4[:], b4_t[:], OP.add)
                nc.sync.dma_start(out_dram[b * 128: b * 128 + blk_rows[b], :], ot[0:blk_rows[b], :])

            prop(1, epi1)
            prop(2, epi2)
            prop(3, epi3)
            prop(4, epi4)

    nc.compile()
    return nc


def make_in_maps(inputs, sched, percore):
    N, NSH, NB = sched["N"], sched["NSH"], sched["NB"]
    wm = prep_weights(inputs, N)
    x = np.asarray(inputs["x"], np.float32)
    in_maps = []
    for c in range(P):
        xs = x[c * NSH: (c + 1) * NSH]
        pad = NB * 128 - NSH
        if pad:
            xs = np.concatenate([xs, np.zeros((pad, 1024), np.float32)], 0)
        xT = xs.astype(BF).reshape(NB, 128, 8, 128).transpose(0, 3, 2, 1).reshape(NB, 128, 1024).copy()
        pc = percore[c]
        dis = np.zeros((128, NB), np.float32)
        dv = pc["dis"]
        for b in range(NB):
            r = min(128, NSH - b * 128)
            dis[0:r, b] = dv[b * 128: b * 128 + r]
        in_maps.append(dict(
            xT=xT, w1=wm["w1"], w2=wm["w2"], w3=wm["w3"], w4=wm["w4"],
            bias4=wm["bias4"], ident=wm["ident"], dis=dis,
            onesr=wm["onesr"], b1r=wm["b1r"], b2r=wm["b2r"], b3r=wm["b3r"],
            sidx=pc["sidx"], oh=pc["oh"], ohT=pc["ohT"],
        ))
    return in_maps


def run(inputs, N=50000, trace=False):
    sched, percore = preprocess(np.asarray(inputs["edge_index"]), N)
    in_maps = make_in_maps(inputs, sched, percore)
    nc = build_nc(sched)
    res = run_bass_kernel_spmd(nc, in_maps, core_ids=list(range(P)), trace=trace)
    out = np.concatenate([res.results[c]["out"] for c in range(P)], axis=0)
    return out, res


def kernel(**inputs):
    out, _ = run(inputs, N=50000)
    return out.astype(np.float32)
